# revision 1
# baseline (speedup 1.0000x reference)
"""Trainium2 Bass kernel for nn_CNN2DImplemented_51994874085714.

conv2d: x (16, 64, 112, 112) f32 * weight (64, 3, 3, 128) -> (16, 128, 112, 112),
3x3, pad=1, stride=1 (weight layout (C_in, kh, kw, C_out), no bias).

Sharding: data-parallel over batch - 2 images per NeuronCore on 8 cores;
weight replicated. Each core runs an implicit-GEMM conv:

  out[b, o, h, w] = sum_{c, dh, dw} x_pad[b, c, h+dh, w+dw] * weight[c, dh, dw, o]

SBUF holds x in 28-output-row strips as [128, S+3, W+2] tiles: partitions
0:64 are x_pad rows (block A), partitions 64:128 the same rows shifted down
one (block B, made by an on-chip copy). One K=128 matmul then contracts
channel taps (dh, dh+1) together:
  pair MM  (dh=0,1): lhsT = [W[:,0,dw,:]; W[:,1,dw,:]]
  single MM (dh=2):  lhsT = [W[:,2,dw,:]; 0]
Six matmuls of N=4*W=448 accumulate each PSUM bank ([O=128, 4, W]), which is
copied to an SBUF staging strip and stored with one DMA per strip.
"""

from contextlib import ExitStack

import numpy as np

MM_DTYPE = "f32"  # "f32" | "f32r" | "bf16"

N_CORES = 8
B, C, H, W, O = 16, 64, 112, 112, 128
B_LOC = B // N_CORES
S = 28  # output rows per strip

_cache = {}


def _build_nc():
    import concourse.mybir as mybir
    import concourse.tile as tile
    from concourse import bacc

    F32 = mybir.dt.float32
    MD = mybir.dt.bfloat16 if MM_DTYPE == "bf16" else F32

    def mmcast(ap):
        return ap.bitcast(mybir.dt.float32r) if MM_DTYPE == "f32r" else ap

    nc = bacc.Bacc("TRN2", target_bir_lowering=False, debug=False,
                   num_devices=N_CORES)
    x_d = nc.declare_dram_parameter("x", [B_LOC, C, H, W], MD, isOutput=False)
    w_d = nc.declare_dram_parameter("weight", [C, 3, 3, O], MD, isOutput=False)
    o_d = nc.declare_dram_parameter("out", [B_LOC, O, H, W], F32, isOutput=True)

    R = S + 3
    Wp = W + 2

    with tile.TileContext(nc) as tc, ExitStack() as ctx:
        wpool = ctx.enter_context(tc.tile_pool(name="weights", bufs=1))
        xpool = ctx.enter_context(tc.tile_pool(name="xstrips", bufs=3))
        spool = ctx.enter_context(tc.tile_pool(name="staging", bufs=3))
        ppool = ctx.enter_context(tc.tile_pool(name="psum", bufs=6, space="PSUM"))

        wpair = []
        wsing = []
        for dw in range(3):
            wp = wpool.tile([128, O], MD, tag=f"wpair{dw}")
            nc.sync.dma_start(wp[0:64, :], w_d[:, 0, dw, :])
            nc.sync.dma_start(wp[64:128, :], w_d[:, 1, dw, :])
            ws = wpool.tile([128, O], MD, tag=f"wsing{dw}")
            nc.vector.memset(ws[64:128, :], 0.0)
            nc.sync.dma_start(ws[0:64, :], w_d[:, 2, dw, :])
            wpair.append(wp)
            wsing.append(ws)

        for b in range(B_LOC):
            for s in range(H // S):
                h0 = s * S
                xb = xpool.tile([128, R, Wp], MD, tag="xs")
                nc.vector.memset(xb[0:64, :, 0], 0.0)
                nc.vector.memset(xb[0:64, :, Wp - 1], 0.0)
                r_lo = max(0, 1 - h0)
                r_hi = min(S + 2, H - h0)
                if r_lo > 0:
                    nc.vector.memset(xb[0:64, 0:r_lo, :], 0.0)
                if r_hi < S + 2:
                    nc.vector.memset(xb[0:64, r_hi + 1:S + 3, :], 0.0)
                nc.sync.dma_start(
                    xb[0:64, r_lo:r_hi + 1, 1:W + 1],
                    x_d[b, :, h0 + r_lo - 1:h0 + r_hi, :],
                )
                nc.vector.tensor_copy(xb[64:128, 0:S + 2, :], xb[0:64, 1:S + 3, :])

                stg = spool.tile([O, S, W], F32, tag="stg")
                for j in range(S // 4):
                    l0 = 4 * j
                    ps = ppool.tile([O, 4, W], F32, tag="ps")
                    for dw in range(3):
                        nc.tensor.matmul(
                            ps[:, :, :],
                            mmcast(wpair[dw][:, :]),
                            mmcast(xb[:, l0:l0 + 4, dw:dw + W]),
                            start=(dw == 0), stop=False,
                        )
                    for dw in range(3):
                        nc.tensor.matmul(
                            ps[:, :, :],
                            mmcast(wsing[dw][:, :]),
                            mmcast(xb[:, l0 + 2:l0 + 6, dw:dw + W]),
                            start=False, stop=(dw == 2),
                        )
                    nc.vector.tensor_copy(stg[:, l0:l0 + 4, :], ps[:, :, :])
                nc.sync.dma_start(o_d[b, :, h0:h0 + S, :], stg[:, :, :])

    nc.compile()
    return nc


def kernel(x: np.ndarray, weight: np.ndarray) -> np.ndarray:
    from concourse.bass_utils import run_bass_kernel_spmd

    if "nc" not in _cache:
        _cache["nc"] = _build_nc()
    nc = _cache["nc"]

    x = np.ascontiguousarray(np.asarray(x, dtype=np.float32))
    w = np.ascontiguousarray(np.asarray(weight, dtype=np.float32))
    if MM_DTYPE == "bf16":
        import ml_dtypes

        x = x.astype(ml_dtypes.bfloat16)
        w = w.astype(ml_dtypes.bfloat16)

    in_maps = [
        {"x": x[i * B_LOC:(i + 1) * B_LOC], "weight": w} for i in range(N_CORES)
    ]
    res = run_bass_kernel_spmd(nc, in_maps, list(range(N_CORES)))
    return np.concatenate(
        [res.results[i]["out"] for i in range(N_CORES)], axis=0
    )


# revision 2
# speedup vs baseline: 1.4467x; 1.4467x over previous
"""Trainium2 Bass kernel for nn_CNN2DImplemented_51994874085714.

conv2d: x (16, 64, 112, 112) f32 * weight (64, 3, 3, 128) -> (16, 128, 112, 112),
3x3, pad=1, stride=1 (weight layout (C_in, kh, kw, C_out), no bias).

Sharding: data-parallel over batch - 2 images per NeuronCore on 8 cores,
weight replicated; each core computes its shard independently (no
collectives) and the host concatenates the per-core outputs.

Per-core kernel (implicit GEMM):
  out[b, o, h, w] = sum_{c, dh, dw} x_pad[b, c, h+dh, w+dw] * weight[c, dh, dw, o]

x and weight DRAM tensors are declared float32r (same bits as f32): the PE
runs fp32r matmuls at 1 column/cycle (4x the fp32 rate) at ~1.6e-4 relative
accuracy. SBUF holds x in 28-output-row strips as [128, S+3, W+2] tiles:
partitions 0:64 are x_pad rows h0..h0+S+2 (block A, DMA'd directly),
partitions 64:128 the same rows shifted down one (block B, produced by a
GPSIMD on-chip move). One K=128 matmul then contracts channel taps
(dh, dh+1) together:
  pair MM  (dh=0,1): lhsT = [W[:,0,dw,:]; W[:,1,dw,:]]
  single MM (dh=2):  lhsT = [W[:,2,dw,:]; 0]
Six matmuls of N=4*W=448 accumulate each PSUM bank ([O=128, 4, W]); DVE
copies banks to an SBUF staging strip which is stored with one DMA.
"""

from contextlib import ExitStack

import numpy as np

N_CORES = 8
B, C, H, W, O = 16, 64, 112, 112, 128
B_LOC = B // N_CORES
S = 28  # output rows per strip

_cache = {}


def _build_nc():
    import concourse.mybir as mybir
    import concourse.tile as tile
    from concourse import bacc

    F32 = mybir.dt.float32
    F32R = mybir.dt.float32r

    nc = bacc.Bacc("TRN2", target_bir_lowering=False, debug=False,
                   num_devices=N_CORES)
    x_d = nc.declare_dram_parameter("x", [B_LOC, C, H, W], F32R, isOutput=False)
    w_d = nc.declare_dram_parameter("weight", [C, 3, 3, O], F32R, isOutput=False)
    o_d = nc.declare_dram_parameter("out", [B_LOC, O, H, W], F32, isOutput=True)

    R = S + 3
    Wp = W + 2
    NS = H // S

    with tile.TileContext(nc) as tc, ExitStack() as ctx:
        wpool = ctx.enter_context(tc.tile_pool(name="weights", bufs=1))
        xpool = ctx.enter_context(tc.tile_pool(name="xstrips", bufs=4))
        spool = ctx.enter_context(tc.tile_pool(name="staging", bufs=3))
        ppool = ctx.enter_context(tc.tile_pool(name="psum", bufs=6, space="PSUM"))

        zrow = wpool.tile([64, O], F32, tag="zrow")
        nc.vector.memset(zrow[:, :], 0.0)
        wpair = []
        wsing = []
        for dw in range(3):
            wp = wpool.tile([128, O], F32R, tag=f"wpair{dw}")
            ws = wpool.tile([128, O], F32R, tag=f"wsing{dw}")
            nc.sync.dma_start(wp[0:64, :], w_d[:, 0, dw, :])
            nc.sync.dma_start(wp[64:128, :], w_d[:, 1, dw, :])
            nc.sync.dma_start(ws[0:64, :], w_d[:, 2, dw, :])
            nc.vector.tensor_copy(ws[64:128, :], zrow[:, :])
            wpair.append(wp)
            wsing.append(ws)

        def load_task(t):
            s, img = t
            h0 = s * S
            xb = xpool.tile([128, R, Wp], F32R, tag="xs")
            xbf = xb.bitcast(F32)
            nc.vector.memset(xbf[0:64, :, 0], 0.0)
            nc.vector.memset(xbf[0:64, :, Wp - 1], 0.0)
            r_lo = max(0, 1 - h0)
            r_hi = min(S + 2, H - h0)
            if r_lo > 0:
                nc.vector.memset(xbf[0:64, 0:r_lo, :], 0.0)
            if r_hi < S + 2:
                nc.vector.memset(xbf[0:64, r_hi + 1:S + 3, :], 0.0)
            nc.sync.dma_start(
                xb[0:64, r_lo:r_hi + 1, 1:W + 1],
                x_d[img, :, h0 + r_lo - 1:h0 + r_hi, :],
            )
            nc.gpsimd.tensor_copy(xb[64:128, 0:S + 2, :], xb[0:64, 1:S + 3, :])
            return xb

        def compute(s, img, xb):
            h0 = s * S
            stg = spool.tile([O, S, W], F32, tag="stg")
            for j in range(S // 4):
                l0 = 4 * j
                ps = ppool.tile([O, 4, W], F32, tag="ps")
                for dw in range(3):
                    nc.tensor.matmul(
                        ps[:, :, :],
                        wpair[dw][:, :],
                        xb[:, l0:l0 + 4, dw:dw + W],
                        start=(dw == 0), stop=False,
                    )
                for dw in range(3):
                    nc.tensor.matmul(
                        ps[:, :, :],
                        wsing[dw][:, :],
                        xb[:, l0 + 2:l0 + 6, dw:dw + W],
                        start=False, stop=(dw == 2),
                    )
                nc.vector.tensor_copy(stg[:, l0:l0 + 4, :], ps[:, :, :])
            nc.sync.dma_start(o_d[img, :, h0:h0 + S, :], stg[:, :, :])

        tasks = [(s, img) for s in range(NS) for img in range(B_LOC)]
        cur = load_task(tasks[0])
        for i, t in enumerate(tasks):
            nxt = load_task(tasks[i + 1]) if i + 1 < len(tasks) else None
            compute(t[0], t[1], cur)
            cur = nxt

    nc.compile()
    return nc


def kernel(x: np.ndarray, weight: np.ndarray) -> np.ndarray:
    from concourse.bass_utils import run_bass_kernel_spmd

    if "nc" not in _cache:
        _cache["nc"] = _build_nc()
    nc = _cache["nc"]

    x = np.ascontiguousarray(np.asarray(x, dtype=np.float32))
    w = np.ascontiguousarray(np.asarray(weight, dtype=np.float32))

    in_maps = [
        {"x": x[i * B_LOC:(i + 1) * B_LOC], "weight": w} for i in range(N_CORES)
    ]
    res = run_bass_kernel_spmd(nc, in_maps, list(range(N_CORES)))
    return np.concatenate(
        [res.results[i]["out"] for i in range(N_CORES)], axis=0
    )
